# revision 8
# baseline (speedup 1.0000x reference)
"""Trainium2 Bass kernel for the Elman-RNN place-cell problem.

Strategy: tensor-parallel over the hidden dimension NG=4096 across 8 cores.
Each core keeps a [4096, 512] column-shard of W_rec resident in SBUF and
computes its 512-column shard of h_{t+1} = relu(x_t + h_t @ W_rec) for the
full batch B=256 each step; a per-step AllGather rebuilds the full hidden
state (transposed layout [NG, B]) on every core.  The decode matmul
(g @ W_dec) is split across cores by NP columns (64 each) and runs on the
TensorE during the AllGather wait.  The encoder (P0 @ W_enc) and the input
projection (v @ W_in) are tiny (<2% FLOPs) and are done on the host as part
of input sharding.
"""
import os
import sys
import functools

sys.path.insert(0, "/opt/trn_rl_repo")

import numpy as np

from concourse import bass, bacc, mybir, tile  # noqa: E402
from concourse import bass_utils  # noqa: E402

B = 256
T = 100
NG = 4096
NP = 512
NCORES = 8
S = NG // NCORES          # 512 hidden columns per core
KT = NG // 128            # 32 contraction tiles
MT = S // 128             # 4 output tiles per core shard
NPS = NP // NCORES        # 64 decode columns per core
FP = mybir.dt.float32

# compute dtype for matmul-facing tensors: "f32r" (fp32 w/ 11-bit mantissa,
# full-rate PE), "f32" (exact, 4x slower PE), or "bf16"
CDTYPE = os.environ.get("RNN_CDTYPE", "f32r")


def _cd():
    return {"f32": mybir.dt.float32,
            "f32r": mybir.dt.float32r,
            "bf16": mybir.dt.bfloat16}[CDTYPE]


HKT = KT // 2  # 16 k-tiles per chunk

# permuted k-tile order: chunk a = each rank's first 2 k-tiles, chunk b =
# each rank's last 2.  An AllGather of each rank's half-shard then lands
# contiguously as SBUF k-tiles [0:16] (a) / [16:32] (b).  wrec/wdec/h0 are
# host-packed in this same order, so the contraction is order-invariant.
K_ORDER = ([4 * r + q for r in range(NCORES) for q in (0, 1)] +
           [4 * r + q for r in range(NCORES) for q in (2, 3)])


def _build(t_steps=T):
    CD = _cd()
    nc = bacc.Bacc("TRN2", target_bir_lowering=False, debug=False,
                   num_devices=NCORES)
    wrec = nc.dram_tensor("wrec", [128, KT * S], CD, kind="ExternalInput")
    wdec = nc.dram_tensor("wdec", [128, KT * NPS], CD, kind="ExternalInput")
    xin = nc.dram_tensor("x", [t_steps, S, B], FP, kind="ExternalInput")
    h0 = nc.dram_tensor("h0", [NG, B], CD, kind="ExternalInput")
    out = nc.dram_tensor("out", [t_steps, 2, 128, NPS], FP,
                         kind="ExternalOutput")

    with tile.TileContext(nc) as tc:
        with tc.tile_pool(name="wpool", bufs=1) as wpool, \
             tc.tile_pool(name="hpool", bufs=2) as hpool, \
             tc.tile_pool(name="xpool", bufs=3) as xpool, \
             tc.tile_pool(name="hnpool", bufs=2) as hnpool, \
             tc.tile_pool(name="decpool", bufs=3) as decpool, \
             tc.tile_pool(name="psr", bufs=4, space="PSUM") as psr, \
             tc.tile_pool(name="psd", bufs=2, space="PSUM") as psd, \
             tc.tile_pool(name="dram_i", bufs=3, space="DRAM") as dram_i, \
             tc.tile_pool(name="dram_o", bufs=3, space="DRAM") as dram_o:

            wrec_sb = wpool.tile([128, KT * S], CD, name="wrec_sb")
            nc.scalar.dma_start(out=wrec_sb[:], in_=wrec[:])
            wdec_sb = wpool.tile([128, KT * NPS], CD, name="wdec_sb")
            nc.scalar.dma_start(out=wdec_sb[:], in_=wdec[:])

            hTa = hpool.tile([128, HKT, B], CD, name="hTa")
            nc.sync.dma_start(
                out=hTa[:],
                in_=h0[0:HKT * 128].rearrange("(k p) b -> p k b", p=128))
            hTb = hpool.tile([128, HKT, B], CD, name="hTb")
            nc.sync.dma_start(
                out=hTb[:],
                in_=h0[HKT * 128:].rearrange("(k p) b -> p k b", p=128))

            def decode(srcs, t):
                # out[t, c*128+p, r*NPS+f] = (g_t @ W_dec)[c*128+p, shard]
                dec_sb = decpool.tile([128, 2, NPS], FP, name="dec_sb")
                for c in range(2):
                    ps = psd.tile([128, NPS], FP, name="ps_dec")
                    for k in range(KT):
                        src = srcs[k // HKT]
                        nc.tensor.matmul(
                            ps[:],
                            src[:, k % HKT, c * 128:(c + 1) * 128],
                            wdec_sb[:, k * NPS:(k + 1) * NPS],
                            start=(k == 0), stop=(k == KT - 1))
                    nc.vector.tensor_copy(dec_sb[:, c, :], ps[:])
                nc.scalar.dma_start(out=out[t].rearrange("c p f -> p c f"),
                                    in_=dec_sb[:])

            def rec_mm(ps_list, src, k_base, k_local, start, stop):
                k = k_base + k_local
                for m in range(MT):
                    nc.tensor.matmul(
                        ps_list[m][:],
                        wrec_sb[:, k * S + m * 128: k * S + (m + 1) * 128],
                        src[:, k_local, :],
                        start=start, stop=stop,
                        skip_group_check=True)

            rg = [list(range(NCORES))]
            for i in range(t_steps):
                x_t = xpool.tile([128, MT, B], FP, name="x_t")
                nc.scalar.dma_start(
                    out=x_t[:],
                    in_=xin[i].rearrange("(m p) b -> p m b", p=128))

                pss = [psr.tile([128, B], FP, name=f"ps_rec{m}", tag="ps_rec")
                       for m in range(MT)]

                # G1: chunk-a k-tiles, k-outer (consume hTa asap)
                for k in range(HKT):
                    rec_mm(pss, hTa, 0, k, start=(k == 0), stop=False)

                # decode of the previous step's state fills the wait for hTb
                if i >= 1:
                    decode((hTa, hTb), i - 1)

                # G2: chunk-b k-tiles, m-outer (finish m-banks early)
                hn = hnpool.tile([128, MT, B], CD, name="hn")
                cc_ia = dram_i.tile([2 * 128, B], CD, name="cc_ia")
                cc_ib = dram_i.tile([2 * 128, B], CD, name="cc_ib")
                cc_oa = dram_o.tile([HKT * 128, B], CD, name="cc_oa",
                                    addr_space="Shared")
                cc_ob = dram_o.tile([HKT * 128, B], CD, name="cc_ob",
                                    addr_space="Shared")
                for m in range(MT):
                    for k in range(HKT):
                        nc.tensor.matmul(
                            pss[m][:],
                            wrec_sb[:, (HKT + k) * S + m * 128:
                                    (HKT + k) * S + (m + 1) * 128],
                            hTb[:, k, :],
                            start=False, stop=(k == HKT - 1),
                            skip_group_check=True)
                    nc.vector.tensor_tensor(hn[:, m, :], pss[m][:],
                                            x_t[:, m, :],
                                            mybir.AluOpType.add)
                    nc.vector.tensor_scalar_max(hn[:, m, :], hn[:, m, :], 0.0)
                    if m == 1:
                        nc.sync.dma_start(
                            out=cc_ia[:].rearrange("(m p) b -> p m b", p=128),
                            in_=hn[:, 0:2, :])
                        nc.gpsimd.collective_compute(
                            "AllGather", mybir.AluOpType.bypass,
                            replica_groups=rg,
                            ins=[cc_ia[:].opt()], outs=[cc_oa[:].opt()])
                    elif m == 3:
                        nc.sync.dma_start(
                            out=cc_ib[:].rearrange("(m p) b -> p m b", p=128),
                            in_=hn[:, 2:4, :])
                        nc.gpsimd.collective_compute(
                            "AllGather", mybir.AluOpType.bypass,
                            replica_groups=rg,
                            ins=[cc_ib[:].opt()], outs=[cc_ob[:].opt()])

                hTa = hpool.tile([128, HKT, B], CD, name="hTa")
                nc.sync.dma_start(
                    out=hTa[:],
                    in_=cc_oa[:].rearrange("(k p) b -> p k b", p=128))
                hTb = hpool.tile([128, HKT, B], CD, name="hTb")
                nc.sync.dma_start(
                    out=hTb[:],
                    in_=cc_ob[:].rearrange("(k p) b -> p k b", p=128))

            decode((hTa, hTb), t_steps - 1)

    nc.compile()
    return nc


@functools.lru_cache(maxsize=1)
def _built():
    return _build()


def _round_f32r(a):
    """Round fp32 to the PE's FP32r format (11-bit mantissa, RNE)."""
    u = np.ascontiguousarray(a, np.float32).view(np.uint32)
    r = (u.astype(np.uint64) + 0x7FF + ((u >> 12) & 1)).astype(np.uint32)
    return (r & np.uint32(0xFFFFF000)).view(np.float32)


def _to_cd(a):
    if CDTYPE == "f32r":
        return _round_f32r(a)
    if CDTYPE == "bf16":
        import ml_dtypes
        return np.ascontiguousarray(a).astype(ml_dtypes.bfloat16)
    return np.ascontiguousarray(a, np.float32)


def _prep_inputs(v, P0, W_enc, W_in, W_rec, W_dec, t_steps=T):
    v = np.asarray(v, np.float32)
    P0 = np.asarray(P0, np.float32)
    W_enc = np.asarray(W_enc, np.float32)
    W_in = np.asarray(W_in, np.float32)
    W_rec = np.asarray(W_rec, np.float32)
    W_dec = np.asarray(W_dec, np.float32)

    # x[t, b, g] = sum_d v[b, t, d] W_in[d, g]; stored transposed [T, NG, B]
    x = (v.reshape(-1, v.shape[-1]) @ W_in).reshape(B, T, NG)
    xT = np.ascontiguousarray(x.transpose(1, 2, 0))  # [T, NG, B]
    h0T = np.ascontiguousarray((P0 @ W_enc).T)       # [NG, B]

    # pack contraction k-tiles in the chunked AllGather order
    wrec_r = W_rec.reshape(KT, 128, NG)[K_ORDER]
    wdec_r = W_dec.reshape(KT, 128, NP)[K_ORDER]
    h0T = np.ascontiguousarray(
        h0T.reshape(KT, 128, B)[K_ORDER].reshape(NG, B))

    in_maps = []
    for r in range(NCORES):
        wrec_core = np.ascontiguousarray(
            wrec_r[:, :, r * S:(r + 1) * S].transpose(1, 0, 2)
        ).reshape(128, KT * S)
        wdec_core = np.ascontiguousarray(
            wdec_r[:, :, r * NPS:(r + 1) * NPS].transpose(1, 0, 2)
        ).reshape(128, KT * NPS)
        x_core = np.ascontiguousarray(xT[:t_steps, r * S:(r + 1) * S, :])
        in_maps.append({
            "wrec": _to_cd(wrec_core),
            "wdec": _to_cd(wdec_core),
            "x": x_core,
            "h0": _to_cd(h0T),
        })
    return in_maps


def _assemble(results, t_steps=T):
    full = np.empty((B, t_steps, NP), np.float32)
    for r in range(NCORES):
        a = results[r]["out"]  # [t_steps, 2, 128, NPS]
        full[:, :, r * NPS:(r + 1) * NPS] = \
            a.reshape(t_steps, B, NPS).transpose(1, 0, 2)
    return full


last_exec_time_ns = None


def kernel(v, P0, W_enc, W_in, W_rec, W_dec):
    global last_exec_time_ns
    nc = _built()
    in_maps = _prep_inputs(v, P0, W_enc, W_in, W_rec, W_dec)

    trace = bool(int(os.environ.get("RNN_TRACE", "0")))
    if trace:
        # NTFF profiling hook (the image's antenv lacks axon_hooks; shim it).
        try:
            import types
            sys.path.insert(0, "/root/.axon_site")
            from trn_agent_boot.trn_boot import _ntff_profile_via_ctypes
            import antenv  # noqa: F401
            if "antenv.axon_hooks" not in sys.modules:
                mod = types.ModuleType("antenv.axon_hooks")
                hook = _ntff_profile_via_ctypes("/opt/axon/libaxon_pjrt.so")
                mod.get_axon_ntff_profile_hook = lambda: hook
                sys.modules["antenv.axon_hooks"] = mod
        except Exception as e:  # pragma: no cover
            print("trace shim failed:", e)

    res = bass_utils.run_bass_kernel_spmd(
        nc, in_maps, core_ids=list(range(NCORES)), trace=trace)
    last_exec_time_ns = res.exec_time_ns
    return _assemble(res.results)


# revision 10
# speedup vs baseline: 1.4459x; 1.4459x over previous
"""Trainium2 Bass kernel for the Elman-RNN place-cell problem.

Strategy: tensor-parallel over the hidden dimension NG=4096 across 8 cores.
Each core keeps a [4096, 512] column-shard of W_rec resident in SBUF and
computes its 512-column shard of h_{t+1} = relu(x_t + h_t @ W_rec) for the
full batch B=256 each step; a per-step AllGather rebuilds the full hidden
state (transposed layout [NG, B]) on every core.  The decode matmul
(g @ W_dec) is split across cores by NP columns (64 each) and runs on the
TensorE during the AllGather wait.  The encoder (P0 @ W_enc) and the input
projection (v @ W_in) are tiny (<2% FLOPs) and are done on the host as part
of input sharding.
"""
import os
import sys
import functools

sys.path.insert(0, "/opt/trn_rl_repo")

import numpy as np

from concourse import bass, bacc, mybir, tile  # noqa: E402
from concourse import bass_utils  # noqa: E402

B = 256
T = 100
NG = 4096
NP = 512
NCORES = 8
S = NG // NCORES          # 512 hidden columns per core
KT = NG // 128            # 32 contraction tiles
MT = S // 128             # 4 output tiles per core shard
NPS = NP // NCORES        # 64 decode columns per core
FP = mybir.dt.float32

# compute dtype for matmul-facing tensors: "f32r" (fp32 w/ 11-bit mantissa,
# full-rate PE), "f32" (exact, 4x slower PE), or "bf16"
CDTYPE = os.environ.get("RNN_CDTYPE", "f32r")


def _cd():
    return {"f32": mybir.dt.float32,
            "f32r": mybir.dt.float32r,
            "bf16": mybir.dt.bfloat16}[CDTYPE]


CH = int(os.environ.get("RNN_CHUNKS", "4"))   # AllGather chunks per step
CPM = MT // CH                                 # m-tiles per chunk
KPC = KT // CH                                 # k-tiles per chunk

# permuted k-tile order: chunk c holds each rank's m-tiles [c*CPM,(c+1)*CPM).
# An AllGather of that slice of every rank lands contiguously as SBUF chunk c.
# wrec/wdec/h0 are host-packed in this order; contraction is order-invariant.
K_ORDER = [4 * r + c * CPM + q
           for c in range(CH) for r in range(NCORES) for q in range(CPM)]


def _build(t_steps=T):
    CD = _cd()
    nc = bacc.Bacc("TRN2", target_bir_lowering=False, debug=False,
                   num_devices=NCORES)
    wrec = nc.dram_tensor("wrec", [128, KT * S], CD, kind="ExternalInput")
    # decode weights stay exact fp32 when compute dtype allows a fp32 view
    DDT = FP if CDTYPE in ("f32", "f32r") else CD
    wdec = nc.dram_tensor("wdec", [128, KT * NPS], DDT, kind="ExternalInput")
    xin = nc.dram_tensor("x", [t_steps, S, B], FP, kind="ExternalInput")
    h0 = nc.dram_tensor("h0", [NG, B], CD, kind="ExternalInput")
    out = nc.dram_tensor("out", [t_steps, 2, 128, NPS], FP,
                         kind="ExternalOutput")

    with tile.TileContext(nc) as tc:
        with tc.tile_pool(name="wpool", bufs=1) as wpool, \
             tc.tile_pool(name="hpool", bufs=2) as hpool, \
             tc.tile_pool(name="xpool", bufs=3) as xpool, \
             tc.tile_pool(name="hnpool", bufs=2) as hnpool, \
             tc.tile_pool(name="decpool", bufs=3) as decpool, \
             tc.tile_pool(name="psr", bufs=4, space="PSUM") as psr, \
             tc.tile_pool(name="psd", bufs=2, space="PSUM") as psd, \
             tc.tile_pool(name="dram_i", bufs=3, space="DRAM") as dram_i, \
             tc.tile_pool(name="dram_o", bufs=3, space="DRAM") as dram_o:

            wrec_sb = wpool.tile([128, KT * S], CD, name="wrec_sb")
            nc.scalar.dma_start(out=wrec_sb[:], in_=wrec[:])
            wdec_sb = wpool.tile([128, KT * NPS], DDT, name="wdec_sb")
            nc.scalar.dma_start(out=wdec_sb[:], in_=wdec[:])

            def new_h_tiles():
                return [hpool.tile([128, KPC, B], CD, name=f"hT{c}",
                                   tag=f"hT{c}") for c in range(CH)]

            hts = new_h_tiles()
            for c in range(CH):
                nc.sync.dma_start(
                    out=hts[c][:],
                    in_=h0[c * KPC * 128:(c + 1) * KPC * 128]
                    .rearrange("(k p) b -> p k b", p=128))

            def decode(srcs, t):
                # out[t, c*128+p, r*NPS+f] = (g_t @ W_dec)[c*128+p, shard]
                # runs in fp32 (2-pass PE) when possible: doubles as a PE
                # keep-warm filler during the AllGather wait.
                dec_sb = decpool.tile([128, 2, NPS], FP, name="dec_sb")
                for c in range(2):
                    ps = psd.tile([128, NPS], FP, name="ps_dec")
                    for k in range(KT):
                        src = srcs[k // KPC]
                        lhsT = src[:, k % KPC, c * 128:(c + 1) * 128]
                        if CDTYPE == "f32r":
                            lhsT = lhsT.bitcast(FP)
                        nc.tensor.matmul(
                            ps[:],
                            lhsT,
                            wdec_sb[:, k * NPS:(k + 1) * NPS],
                            start=(k == 0), stop=(k == KT - 1))
                    nc.vector.tensor_copy(dec_sb[:, c, :], ps[:])
                nc.scalar.dma_start(out=out[t].rearrange("c p f -> p c f"),
                                    in_=dec_sb[:])

            rg = [list(range(NCORES))]
            for i in range(t_steps):
                x_t = xpool.tile([128, MT, B], FP, name="x_t")
                nc.scalar.dma_start(
                    out=x_t[:],
                    in_=xin[i].rearrange("(m p) b -> p m b", p=128))

                pss = [psr.tile([128, B], FP, name=f"ps_rec{m}", tag="ps_rec")
                       for m in range(MT)]

                hn = hnpool.tile([128, MT, B], CD, name="hn")
                cc_is = [dram_i.tile([CPM * 128, B], CD, name=f"cc_i{c}",
                                     tag=f"cc_i{c}") for c in range(CH)]
                cc_os = [dram_o.tile([KPC * 128, B], CD, name=f"cc_o{c}",
                                     tag=f"cc_o{c}", addr_space="Shared")
                         for c in range(CH)]

                # consume chunks 0..CH-2 k-outer (start as each chunk lands)
                for c in range(CH - 1):
                    for kl in range(KPC):
                        k = c * KPC + kl
                        for m in range(MT):
                            nc.tensor.matmul(
                                pss[m][:],
                                wrec_sb[:, k * S + m * 128:
                                        k * S + (m + 1) * 128],
                                hts[c][:, kl, :],
                                start=(k == 0), stop=False,
                                skip_group_check=True)
                    if c == 0 and i >= 1:
                        # previous state decode fills the chunk-1 wait
                        decode(hts, i - 1)

                # last chunk m-outer: close banks early, send chunks asap
                c = CH - 1
                for m in range(MT):
                    for kl in range(KPC):
                        k = c * KPC + kl
                        nc.tensor.matmul(
                            pss[m][:],
                            wrec_sb[:, k * S + m * 128: k * S + (m + 1) * 128],
                            hts[c][:, kl, :],
                            start=False, stop=(kl == KPC - 1),
                            skip_group_check=True)
                    nc.vector.tensor_tensor(hn[:, m, :], pss[m][:],
                                            x_t[:, m, :],
                                            mybir.AluOpType.add)
                    nc.vector.tensor_scalar_max(hn[:, m, :], hn[:, m, :], 0.0)
                    if (m + 1) % CPM == 0:
                        cs = (m + 1) // CPM - 1   # chunk just completed
                        nc.sync.dma_start(
                            out=cc_is[cs][:].rearrange("(m p) b -> p m b",
                                                       p=128),
                            in_=hn[:, cs * CPM:(cs + 1) * CPM, :])
                        nc.gpsimd.collective_compute(
                            "AllGather", mybir.AluOpType.bypass,
                            replica_groups=rg,
                            ins=[cc_is[cs][:].opt()],
                            outs=[cc_os[cs][:].opt()])

                hts = new_h_tiles()
                for c in range(CH):
                    nc.sync.dma_start(
                        out=hts[c][:],
                        in_=cc_os[c][:].rearrange("(k p) b -> p k b", p=128))

            decode(hts, t_steps - 1)

    nc.compile()
    return nc


@functools.lru_cache(maxsize=1)
def _built():
    return _build()


def _round_f32r(a):
    """Round fp32 to the PE's FP32r format (11-bit mantissa, RNE)."""
    u = np.ascontiguousarray(a, np.float32).view(np.uint32)
    r = (u.astype(np.uint64) + 0x7FF + ((u >> 12) & 1)).astype(np.uint32)
    return (r & np.uint32(0xFFFFF000)).view(np.float32)


def _to_cd(a):
    if CDTYPE == "f32r":
        return _round_f32r(a)
    if CDTYPE == "bf16":
        import ml_dtypes
        return np.ascontiguousarray(a).astype(ml_dtypes.bfloat16)
    return np.ascontiguousarray(a, np.float32)


def _prep_inputs(v, P0, W_enc, W_in, W_rec, W_dec, t_steps=T):
    v = np.asarray(v, np.float32)
    P0 = np.asarray(P0, np.float32)
    W_enc = np.asarray(W_enc, np.float32)
    W_in = np.asarray(W_in, np.float32)
    W_rec = np.asarray(W_rec, np.float32)
    W_dec = np.asarray(W_dec, np.float32)

    # x[t, b, g] = sum_d v[b, t, d] W_in[d, g]; stored transposed [T, NG, B]
    x = (v.reshape(-1, v.shape[-1]) @ W_in).reshape(B, T, NG)
    xT = np.ascontiguousarray(x.transpose(1, 2, 0))  # [T, NG, B]
    h0T = np.ascontiguousarray((P0 @ W_enc).T)       # [NG, B]

    # pack contraction k-tiles in the chunked AllGather order
    wrec_r = W_rec.reshape(KT, 128, NG)[K_ORDER]
    wdec_r = W_dec.reshape(KT, 128, NP)[K_ORDER]
    h0T = np.ascontiguousarray(
        h0T.reshape(KT, 128, B)[K_ORDER].reshape(NG, B))

    in_maps = []
    for r in range(NCORES):
        wrec_core = np.ascontiguousarray(
            wrec_r[:, :, r * S:(r + 1) * S].transpose(1, 0, 2)
        ).reshape(128, KT * S)
        wdec_core = np.ascontiguousarray(
            wdec_r[:, :, r * NPS:(r + 1) * NPS].transpose(1, 0, 2)
        ).reshape(128, KT * NPS)
        x_core = np.ascontiguousarray(xT[:t_steps, r * S:(r + 1) * S, :])
        in_maps.append({
            "wrec": _to_cd(wrec_core),
            # decode weights stay exact fp32 for f32/f32r compute modes
            "wdec": (wdec_core if CDTYPE in ("f32", "f32r")
                     else _to_cd(wdec_core)),
            "x": x_core,
            "h0": _to_cd(h0T),
        })
    return in_maps


def _assemble(results, t_steps=T):
    full = np.empty((B, t_steps, NP), np.float32)
    for r in range(NCORES):
        a = results[r]["out"]  # [t_steps, 2, 128, NPS]
        full[:, :, r * NPS:(r + 1) * NPS] = \
            a.reshape(t_steps, B, NPS).transpose(1, 0, 2)
    return full


last_exec_time_ns = None


def kernel(v, P0, W_enc, W_in, W_rec, W_dec):
    global last_exec_time_ns
    nc = _built()
    in_maps = _prep_inputs(v, P0, W_enc, W_in, W_rec, W_dec)

    trace = bool(int(os.environ.get("RNN_TRACE", "0")))
    if trace:
        # NTFF profiling hook (the image's antenv lacks axon_hooks; shim it).
        try:
            import types
            sys.path.insert(0, "/root/.axon_site")
            from trn_agent_boot.trn_boot import _ntff_profile_via_ctypes
            import antenv  # noqa: F401
            if "antenv.axon_hooks" not in sys.modules:
                mod = types.ModuleType("antenv.axon_hooks")
                hook = _ntff_profile_via_ctypes("/opt/axon/libaxon_pjrt.so")
                mod.get_axon_ntff_profile_hook = lambda: hook
                sys.modules["antenv.axon_hooks"] = mod
        except Exception as e:  # pragma: no cover
            print("trace shim failed:", e)

    res = bass_utils.run_bass_kernel_spmd(
        nc, in_maps, core_ids=list(range(NCORES)), trace=trace)
    last_exec_time_ns = res.exec_time_ns
    return _assemble(res.results)


# revision 11
# speedup vs baseline: 1.5377x; 1.0635x over previous
"""Trainium2 Bass kernel for the Elman-RNN place-cell problem.

Strategy: tensor-parallel over the hidden dimension NG=4096 across 8 cores.
Each core keeps a [4096, 512] column-shard of W_rec resident in SBUF and
computes its 512-column shard of h_{t+1} = relu(x_t + h_t @ W_rec) for the
full batch B=256 each step; a per-step AllGather rebuilds the full hidden
state (transposed layout [NG, B]) on every core.  The decode matmul
(g @ W_dec) is split across cores by NP columns (64 each) and runs on the
TensorE during the AllGather wait.  The encoder (P0 @ W_enc) and the input
projection (v @ W_in) are tiny (<2% FLOPs) and are done on the host as part
of input sharding.
"""
import os
import sys
import functools

sys.path.insert(0, "/opt/trn_rl_repo")

import numpy as np

from concourse import bass, bacc, mybir, tile  # noqa: E402
from concourse import bass_utils  # noqa: E402

B = 256
T = 100
NG = 4096
NP = 512
NCORES = 8
S = NG // NCORES          # 512 hidden columns per core
KT = NG // 128            # 32 contraction tiles
MT = S // 128             # 4 output tiles per core shard
NPS = NP // NCORES        # 64 decode columns per core
FP = mybir.dt.float32

# compute dtype for matmul-facing tensors: "f32r" (fp32 w/ 11-bit mantissa,
# full-rate PE), "f32" (exact, 4x slower PE), or "bf16"
CDTYPE = os.environ.get("RNN_CDTYPE", "f32r")


def _cd():
    return {"f32": mybir.dt.float32,
            "f32r": mybir.dt.float32r,
            "bf16": mybir.dt.bfloat16}[CDTYPE]


CH = int(os.environ.get("RNN_CHUNKS", "4"))   # AllGather chunks per step
CPM = MT // CH                                 # m-tiles per chunk
KPC = KT // CH                                 # k-tiles per chunk

# permuted k-tile order: chunk c holds each rank's m-tiles [c*CPM,(c+1)*CPM).
# An AllGather of that slice of every rank lands contiguously as SBUF chunk c.
# wrec/wdec/h0 are host-packed in this order; contraction is order-invariant.
K_ORDER = [4 * r + c * CPM + q
           for c in range(CH) for r in range(NCORES) for q in range(CPM)]


def _build(t_steps=T):
    CD = _cd()
    nc = bacc.Bacc("TRN2", target_bir_lowering=False, debug=False,
                   num_devices=NCORES)
    wrec = nc.dram_tensor("wrec", [128, KT * S], CD, kind="ExternalInput")
    # decode weights stay exact fp32 when compute dtype allows a fp32 view
    DDT = FP if CDTYPE in ("f32", "f32r") else CD
    wdec = nc.dram_tensor("wdec", [128, KT * NPS], DDT, kind="ExternalInput")
    xin = nc.dram_tensor("x", [t_steps, S, B], FP, kind="ExternalInput")
    h0 = nc.dram_tensor("h0", [NG, B], CD, kind="ExternalInput")
    out = nc.dram_tensor("out", [t_steps, 2, 128, NPS], FP,
                         kind="ExternalOutput")

    with tile.TileContext(nc) as tc:
        with tc.tile_pool(name="wpool", bufs=1) as wpool, \
             tc.tile_pool(name="hpool", bufs=2) as hpool, \
             tc.tile_pool(name="xpool", bufs=3) as xpool, \
             tc.tile_pool(name="hnpool", bufs=2) as hnpool, \
             tc.tile_pool(name="decpool", bufs=3) as decpool, \
             tc.tile_pool(name="psr", bufs=4, space="PSUM") as psr, \
             tc.tile_pool(name="psd", bufs=2, space="PSUM") as psd, \
             tc.tile_pool(name="dram_i", bufs=3, space="DRAM") as dram_i, \
             tc.tile_pool(name="dram_o", bufs=3, space="DRAM") as dram_o:

            wrec_sb = wpool.tile([128, KT * S], CD, name="wrec_sb")
            nc.scalar.dma_start(out=wrec_sb[:], in_=wrec[:])
            wdec_sb = wpool.tile([128, KT * NPS], DDT, name="wdec_sb")
            nc.scalar.dma_start(out=wdec_sb[:], in_=wdec[:])

            # h state lives in BCH=4 quarter tiles so the post-AllGather
            # bounce DMAs land piecewise and matmuls start on quarter 0.
            BCH = 4
            KPB = KT // BCH

            def new_h_tiles():
                return [hpool.tile([128, KPB, B], CD, name=f"hT{c}",
                                   tag=f"hT{c}") for c in range(BCH)]

            hts = new_h_tiles()
            for c in range(BCH):
                nc.sync.dma_start(
                    out=hts[c][:],
                    in_=h0[c * KPB * 128:(c + 1) * KPB * 128]
                    .rearrange("(k p) b -> p k b", p=128))

            def decode(srcs, t):
                # out[t, c*128+p, r*NPS+f] = (g_t @ W_dec)[c*128+p, shard]
                # runs in fp32 (2-pass PE) when possible: doubles as a PE
                # keep-warm filler during the AllGather wait.
                dec_sb = decpool.tile([128, 2, NPS], FP, name="dec_sb")
                for c in range(2):
                    ps = psd.tile([128, NPS], FP, name="ps_dec")
                    for k in range(KT):
                        src = srcs[k // KPB]
                        lhsT = src[:, k % KPB, c * 128:(c + 1) * 128]
                        if CDTYPE == "f32r":
                            lhsT = lhsT.bitcast(FP)
                        nc.tensor.matmul(
                            ps[:],
                            lhsT,
                            wdec_sb[:, k * NPS:(k + 1) * NPS],
                            start=(k == 0), stop=(k == KT - 1))
                    nc.vector.tensor_copy(dec_sb[:, c, :], ps[:])
                nc.scalar.dma_start(out=out[t].rearrange("c p f -> p c f"),
                                    in_=dec_sb[:])

            rg = [list(range(NCORES))]
            for i in range(t_steps):
                x_t = xpool.tile([128, MT, B], FP, name="x_t")
                nc.scalar.dma_start(
                    out=x_t[:],
                    in_=xin[i].rearrange("(m p) b -> p m b", p=128))

                pss = [psr.tile([128, B], FP, name=f"ps_rec{m}", tag="ps_rec")
                       for m in range(MT)]

                hn = hnpool.tile([128, MT, B], CD, name="hn")
                cc_is = [dram_i.tile([CPM * 128, B], CD, name=f"cc_i{c}",
                                     tag=f"cc_i{c}") for c in range(CH)]
                cc_os = [dram_o.tile([KPC * 128, B], CD, name=f"cc_o{c}",
                                     tag=f"cc_o{c}", addr_space="Shared")
                         for c in range(CH)]

                # consume quarters 0..BCH-2 k-outer (start as each lands)
                for c in range(BCH - 1):
                    for kl in range(KPB):
                        k = c * KPB + kl
                        for m in range(MT):
                            nc.tensor.matmul(
                                pss[m][:],
                                wrec_sb[:, k * S + m * 128:
                                        k * S + (m + 1) * 128],
                                hts[c][:, kl, :],
                                start=(k == 0), stop=False,
                                skip_group_check=True)
                    if CH > 1 and c == BCH // CH - 1 and i >= 1:
                        # previous state decode fills the next-chunk wait
                        decode(hts, i - 1)

                # last quarter m-outer: close banks early, send chunks asap
                c = BCH - 1
                for m in range(MT):
                    for kl in range(KPB):
                        k = c * KPB + kl
                        nc.tensor.matmul(
                            pss[m][:],
                            wrec_sb[:, k * S + m * 128: k * S + (m + 1) * 128],
                            hts[c][:, kl, :],
                            start=False, stop=(kl == KPB - 1),
                            skip_group_check=True)
                    nc.vector.tensor_tensor(hn[:, m, :], pss[m][:],
                                            x_t[:, m, :],
                                            mybir.AluOpType.add)
                    nc.vector.tensor_scalar_max(hn[:, m, :], hn[:, m, :], 0.0)
                    if (m + 1) % CPM == 0:
                        cs = (m + 1) // CPM - 1   # chunk just completed
                        nc.sync.dma_start(
                            out=cc_is[cs][:].rearrange("(m p) b -> p m b",
                                                       p=128),
                            in_=hn[:, cs * CPM:(cs + 1) * CPM, :])
                        nc.gpsimd.collective_compute(
                            "AllGather", mybir.AluOpType.bypass,
                            replica_groups=rg,
                            ins=[cc_is[cs][:].opt()],
                            outs=[cc_os[cs][:].opt()])

                if CH == 1 and i >= 1:
                    # single-AG mode: decode fills the AllGather wait
                    decode(hts, i - 1)

                hts = new_h_tiles()
                qpc = BCH // CH   # bounce quarters per AG chunk
                for c in range(BCH):
                    a, q = c // qpc, c % qpc
                    nc.sync.dma_start(
                        out=hts[c][:],
                        in_=cc_os[a][q * KPB * 128:(q + 1) * KPB * 128]
                        .rearrange("(k p) b -> p k b", p=128))

            decode(hts, t_steps - 1)

    nc.compile()
    return nc


@functools.lru_cache(maxsize=1)
def _built():
    return _build()


def _round_f32r(a):
    """Round fp32 to the PE's FP32r format (11-bit mantissa, RNE)."""
    u = np.ascontiguousarray(a, np.float32).view(np.uint32)
    r = (u.astype(np.uint64) + 0x7FF + ((u >> 12) & 1)).astype(np.uint32)
    return (r & np.uint32(0xFFFFF000)).view(np.float32)


def _to_cd(a):
    if CDTYPE == "f32r":
        return _round_f32r(a)
    if CDTYPE == "bf16":
        import ml_dtypes
        return np.ascontiguousarray(a).astype(ml_dtypes.bfloat16)
    return np.ascontiguousarray(a, np.float32)


def _prep_inputs(v, P0, W_enc, W_in, W_rec, W_dec, t_steps=T):
    v = np.asarray(v, np.float32)
    P0 = np.asarray(P0, np.float32)
    W_enc = np.asarray(W_enc, np.float32)
    W_in = np.asarray(W_in, np.float32)
    W_rec = np.asarray(W_rec, np.float32)
    W_dec = np.asarray(W_dec, np.float32)

    # x[t, b, g] = sum_d v[b, t, d] W_in[d, g]; stored transposed [T, NG, B]
    x = (v.reshape(-1, v.shape[-1]) @ W_in).reshape(B, T, NG)
    xT = np.ascontiguousarray(x.transpose(1, 2, 0))  # [T, NG, B]
    h0T = np.ascontiguousarray((P0 @ W_enc).T)       # [NG, B]

    # pack contraction k-tiles in the chunked AllGather order
    wrec_r = W_rec.reshape(KT, 128, NG)[K_ORDER]
    wdec_r = W_dec.reshape(KT, 128, NP)[K_ORDER]
    h0T = np.ascontiguousarray(
        h0T.reshape(KT, 128, B)[K_ORDER].reshape(NG, B))

    in_maps = []
    for r in range(NCORES):
        wrec_core = np.ascontiguousarray(
            wrec_r[:, :, r * S:(r + 1) * S].transpose(1, 0, 2)
        ).reshape(128, KT * S)
        wdec_core = np.ascontiguousarray(
            wdec_r[:, :, r * NPS:(r + 1) * NPS].transpose(1, 0, 2)
        ).reshape(128, KT * NPS)
        x_core = np.ascontiguousarray(xT[:t_steps, r * S:(r + 1) * S, :])
        in_maps.append({
            "wrec": _to_cd(wrec_core),
            # decode weights stay exact fp32 for f32/f32r compute modes
            "wdec": (wdec_core if CDTYPE in ("f32", "f32r")
                     else _to_cd(wdec_core)),
            "x": x_core,
            "h0": _to_cd(h0T),
        })
    return in_maps


def _assemble(results, t_steps=T):
    full = np.empty((B, t_steps, NP), np.float32)
    for r in range(NCORES):
        a = results[r]["out"]  # [t_steps, 2, 128, NPS]
        full[:, :, r * NPS:(r + 1) * NPS] = \
            a.reshape(t_steps, B, NPS).transpose(1, 0, 2)
    return full


last_exec_time_ns = None


def kernel(v, P0, W_enc, W_in, W_rec, W_dec):
    global last_exec_time_ns
    nc = _built()
    in_maps = _prep_inputs(v, P0, W_enc, W_in, W_rec, W_dec)

    trace = bool(int(os.environ.get("RNN_TRACE", "0")))
    if trace:
        # NTFF profiling hook (the image's antenv lacks axon_hooks; shim it).
        try:
            import types
            sys.path.insert(0, "/root/.axon_site")
            from trn_agent_boot.trn_boot import _ntff_profile_via_ctypes
            import antenv  # noqa: F401
            if "antenv.axon_hooks" not in sys.modules:
                mod = types.ModuleType("antenv.axon_hooks")
                hook = _ntff_profile_via_ctypes("/opt/axon/libaxon_pjrt.so")
                mod.get_axon_ntff_profile_hook = lambda: hook
                sys.modules["antenv.axon_hooks"] = mod
        except Exception as e:  # pragma: no cover
            print("trace shim failed:", e)

    res = bass_utils.run_bass_kernel_spmd(
        nc, in_maps, core_ids=list(range(NCORES)), trace=trace)
    last_exec_time_ns = res.exec_time_ns
    return _assemble(res.results)


# revision 15
# speedup vs baseline: 1.9292x; 1.2546x over previous
"""Trainium2 Bass kernel for the Elman-RNN place-cell problem.

Strategy: tensor-parallel over the hidden dimension NG=4096 across 8 cores.
Each core keeps a [4096, 512] column-shard of W_rec resident in SBUF and
computes its 512-column shard of h_{t+1} = relu(x_t + h_t @ W_rec) for the
full batch B=256 each step; a per-step AllGather rebuilds the full hidden
state (transposed layout [NG, B]) on every core.  The decode matmul
(g @ W_dec) is split across cores by NP columns (64 each) and runs on the
TensorE during the AllGather wait.  The encoder (P0 @ W_enc) and the input
projection (v @ W_in) are tiny (<2% FLOPs) and are done on the host as part
of input sharding.
"""
import os
import sys
import functools

sys.path.insert(0, "/opt/trn_rl_repo")

import numpy as np

from concourse import bass, bacc, mybir, tile  # noqa: E402
from concourse import bass_utils  # noqa: E402

B = 256
T = 100
NG = 4096
NP = 512
NCORES = 8
S = NG // NCORES          # 512 hidden columns per core
KT = NG // 128            # 32 contraction tiles
MT = S // 128             # 4 output tiles per core shard
NPS = NP // NCORES        # 64 decode columns per core
FP = mybir.dt.float32

# compute dtype for matmul-facing tensors: "f32r" (fp32 w/ 11-bit mantissa,
# full-rate PE), "f32" (exact, 4x slower PE), or "bf16"
CDTYPE = os.environ.get("RNN_CDTYPE", "f32r")


def _cd():
    return {"f32": mybir.dt.float32,
            "f32r": mybir.dt.float32r,
            "bf16": mybir.dt.bfloat16}[CDTYPE]


# one AllGather per batch-half per step; k-tiles stay in natural order
K_ORDER = list(range(KT))


def _build(t_steps=T):
    CD = _cd()
    nc = bacc.Bacc("TRN2", target_bir_lowering=False, debug=False,
                   num_devices=NCORES)
    wrec = nc.dram_tensor("wrec", [128, KT * S], CD, kind="ExternalInput")
    # decode weights stay exact fp32 when compute dtype allows a fp32 view
    DDT = FP if CDTYPE in ("f32", "f32r") else CD
    wdec = nc.dram_tensor("wdec", [128, KT * NPS], DDT, kind="ExternalInput")
    xin = nc.dram_tensor("x", [t_steps, S, B], FP, kind="ExternalInput")
    h0 = nc.dram_tensor("h0", [NG, B], CD, kind="ExternalInput")
    out = nc.dram_tensor("out", [t_steps, 2, 128, NPS], FP,
                         kind="ExternalOutput")

    with tile.TileContext(nc) as tc:
        with tc.tile_pool(name="wpool", bufs=1) as wpool, \
             tc.tile_pool(name="hpool", bufs=2) as hpool, \
             tc.tile_pool(name="xpool", bufs=3) as xpool, \
             tc.tile_pool(name="hnpool", bufs=2) as hnpool, \
             tc.tile_pool(name="decpool", bufs=3) as decpool, \
             tc.tile_pool(name="psr", bufs=1, space="PSUM") as psr, \
             tc.tile_pool(name="psd", bufs=2, space="PSUM") as psd, \
             tc.tile_pool(name="dram_i", bufs=3, space="DRAM") as dram_i, \
             tc.tile_pool(name="dram_o", bufs=3, space="DRAM") as dram_o:

            wrec_sb = wpool.tile([128, KT * S], CD, name="wrec_sb")
            nc.scalar.dma_start(out=wrec_sb[:], in_=wrec[:])
            wdec_sb = wpool.tile([128, KT * NPS], DDT, name="wdec_sb")
            nc.scalar.dma_start(out=wdec_sb[:], in_=wdec[:])

            # The batch is split in two independent halves (BH=128 each);
            # their recurrences interleave so one half's AllGather hides
            # under the other half's matmuls.  h state lives in BCH=4
            # quarter tiles per half so the post-AllGather bounce DMAs land
            # piecewise and matmuls start on quarter 0.
            BCH = 4
            KPB = KT // BCH
            BH = B // 2

            def new_h_tiles(h):
                return [hpool.tile([128, KPB, BH], CD, name=f"hT{h}_{c}",
                                   tag=f"hT{h}_{c}") for c in range(BCH)]

            hts = [new_h_tiles(0), new_h_tiles(1)]
            for h in range(2):
                for c in range(BCH):
                    nc.sync.dma_start(
                        out=hts[h][c][:],
                        in_=h0[c * KPB * 128:(c + 1) * KPB * 128,
                               h * BH:(h + 1) * BH]
                        .rearrange("(k p) b -> p k b", p=128))

            def decode(srcs, h, t):
                # out[t, h*128+p, r*NPS+f] = (g_t @ W_dec)[h*128+p, shard]
                dec_sb = decpool.tile([128, NPS], FP, name="dec_sb")
                ps = psd.tile([128, NPS], FP, name="ps_dec")
                for k in range(KT):
                    nc.tensor.matmul(
                        ps[:],
                        srcs[k // KPB][:, k % KPB, :],
                        wdec_sb[:, k * NPS:(k + 1) * NPS],
                        start=(k == 0), stop=(k == KT - 1))
                nc.vector.tensor_copy(dec_sb[:], ps[:])
                nc.scalar.dma_start(out=out[t, h], in_=dec_sb[:])

            rg = [list(range(NCORES))]
            for i in range(t_steps):
                for h in range(2):
                    x_t = xpool.tile([128, MT, BH], FP, name=f"x_t{h}",
                                     tag=f"x_t{h}")
                    nc.scalar.dma_start(
                        out=x_t[:],
                        in_=xin[i][:, h * BH:(h + 1) * BH]
                        .rearrange("(m p) b -> p m b", p=128))

                    # two m-banks share one PSUM bank (padded per tile)
                    pss = [psr.tile([128, 2, BH], FP, name=f"ps{h}_{p}",
                                    tag=f"ps{h}_{p}") for p in range(2)]

                    hn = hnpool.tile([128, MT, BH], CD, name=f"hn{h}",
                                     tag=f"hn{h}")
                    cc_i = dram_i.tile([S, BH], CD, name=f"cc_i{h}",
                                       tag=f"cc_i{h}")
                    cc_o = dram_o.tile([NG, BH], CD, name=f"cc_o{h}",
                                       tag=f"cc_o{h}", addr_space="Shared")

                    # quarters 0..BCH-2 k-outer (start as each lands)
                    for c in range(BCH - 1):
                        for kl in range(KPB):
                            k = c * KPB + kl
                            for m in range(MT):
                                # start=True zeroes the whole 2KB bank, so
                                # only the pair's first m-group issues it
                                nc.tensor.matmul(
                                    pss[m // 2][:, m % 2, :],
                                    wrec_sb[:, k * S + m * 128:
                                            k * S + (m + 1) * 128],
                                    hts[h][c][:, kl, :],
                                    start=(k == 0 and m % 2 == 0), stop=False,
                                    skip_group_check=True)

                    # last quarter m-outer: close banks early, send asap
                    c = BCH - 1
                    for m in range(MT):
                        for kl in range(KPB):
                            k = c * KPB + kl
                            nc.tensor.matmul(
                                pss[m // 2][:, m % 2, :],
                                wrec_sb[:, k * S + m * 128:
                                        k * S + (m + 1) * 128],
                                hts[h][c][:, kl, :],
                                start=False, stop=(kl == KPB - 1),
                                skip_group_check=True)
                        nc.vector.tensor_tensor(hn[:, m, :],
                                                pss[m // 2][:, m % 2, :],
                                                x_t[:, m, :],
                                                mybir.AluOpType.add)
                        nc.vector.tensor_scalar_max(hn[:, m, :],
                                                    hn[:, m, :], 0.0)
                    nc.sync.dma_start(
                        out=cc_i[:].rearrange("(m p) b -> p m b", p=128),
                        in_=hn[:])
                    nc.gpsimd.collective_compute(
                        "AllGather", mybir.AluOpType.bypass,
                        replica_groups=rg,
                        ins=[cc_i[:].opt()], outs=[cc_o[:].opt()])

                    # decode of this half's previous state fills the AG wait
                    if i >= 1:
                        decode(hts[h], h, i - 1)

                    hts[h] = new_h_tiles(h)
                    for c in range(BCH):
                        nc.sync.dma_start(
                            out=hts[h][c][:],
                            in_=cc_o[c * KPB * 128:(c + 1) * KPB * 128]
                            .rearrange("(k p) b -> p k b", p=128))

            for h in range(2):
                decode(hts[h], h, t_steps - 1)

    nc.compile()
    return nc


@functools.lru_cache(maxsize=1)
def _built():
    return _build()


def _round_f32r(a):
    """Round fp32 to the PE's FP32r format (11-bit mantissa, RNE)."""
    u = np.ascontiguousarray(a, np.float32).view(np.uint32)
    r = (u.astype(np.uint64) + 0x7FF + ((u >> 12) & 1)).astype(np.uint32)
    return (r & np.uint32(0xFFFFF000)).view(np.float32)


def _to_cd(a):
    if CDTYPE == "f32r":
        return _round_f32r(a)
    if CDTYPE == "bf16":
        import ml_dtypes
        return np.ascontiguousarray(a).astype(ml_dtypes.bfloat16)
    return np.ascontiguousarray(a, np.float32)


def _prep_inputs(v, P0, W_enc, W_in, W_rec, W_dec, t_steps=T):
    v = np.asarray(v, np.float32)
    P0 = np.asarray(P0, np.float32)
    W_enc = np.asarray(W_enc, np.float32)
    W_in = np.asarray(W_in, np.float32)
    W_rec = np.asarray(W_rec, np.float32)
    W_dec = np.asarray(W_dec, np.float32)

    # x[t, b, g] = sum_d v[b, t, d] W_in[d, g]; stored transposed [T, NG, B]
    x = (v.reshape(-1, v.shape[-1]) @ W_in).reshape(B, T, NG)
    xT = np.ascontiguousarray(x.transpose(1, 2, 0))  # [T, NG, B]
    h0T = np.ascontiguousarray((P0 @ W_enc).T)       # [NG, B]

    # pack contraction k-tiles in the chunked AllGather order
    wrec_r = W_rec.reshape(KT, 128, NG)[K_ORDER]
    wdec_r = W_dec.reshape(KT, 128, NP)[K_ORDER]
    h0T = np.ascontiguousarray(
        h0T.reshape(KT, 128, B)[K_ORDER].reshape(NG, B))

    in_maps = []
    for r in range(NCORES):
        wrec_core = np.ascontiguousarray(
            wrec_r[:, :, r * S:(r + 1) * S].transpose(1, 0, 2)
        ).reshape(128, KT * S)
        wdec_core = np.ascontiguousarray(
            wdec_r[:, :, r * NPS:(r + 1) * NPS].transpose(1, 0, 2)
        ).reshape(128, KT * NPS)
        x_core = np.ascontiguousarray(xT[:t_steps, r * S:(r + 1) * S, :])
        in_maps.append({
            "wrec": _to_cd(wrec_core),
            # decode weights stay exact fp32 for f32/f32r compute modes
            "wdec": (wdec_core if CDTYPE in ("f32", "f32r")
                     else _to_cd(wdec_core)),
            "x": x_core,
            "h0": _to_cd(h0T),
        })
    return in_maps


def _assemble(results, t_steps=T):
    full = np.empty((B, t_steps, NP), np.float32)
    for r in range(NCORES):
        a = results[r]["out"]  # [t_steps, 2, 128, NPS]
        full[:, :, r * NPS:(r + 1) * NPS] = \
            a.reshape(t_steps, B, NPS).transpose(1, 0, 2)
    return full


last_exec_time_ns = None


def kernel(v, P0, W_enc, W_in, W_rec, W_dec):
    global last_exec_time_ns
    nc = _built()
    in_maps = _prep_inputs(v, P0, W_enc, W_in, W_rec, W_dec)

    trace = bool(int(os.environ.get("RNN_TRACE", "0")))
    if trace:
        # NTFF profiling hook (the image's antenv lacks axon_hooks; shim it).
        try:
            import types
            sys.path.insert(0, "/root/.axon_site")
            from trn_agent_boot.trn_boot import _ntff_profile_via_ctypes
            import antenv  # noqa: F401
            if "antenv.axon_hooks" not in sys.modules:
                mod = types.ModuleType("antenv.axon_hooks")
                hook = _ntff_profile_via_ctypes("/opt/axon/libaxon_pjrt.so")
                mod.get_axon_ntff_profile_hook = lambda: hook
                sys.modules["antenv.axon_hooks"] = mod
        except Exception as e:  # pragma: no cover
            print("trace shim failed:", e)

    res = bass_utils.run_bass_kernel_spmd(
        nc, in_maps, core_ids=list(range(NCORES)), trace=trace)
    last_exec_time_ns = res.exec_time_ns
    return _assemble(res.results)
